# revision 18
# baseline (speedup 1.0000x reference)
"""DepthGatedModule kernel v3 for 8 Trainium2 NeuronCores (Bass/Tile).

vs v2:
- rhs/xt linears computed once per token slice and AllGathered (packed
  f32 rhs + bf16 xt in one DRAM buffer per 384-token sub-slice) instead
  of recomputed 8x. Key loop streams gathered tiles; no in-loop linears.
- softmax denominator via one ones-stationary matmul [1 x 384] per
  (chunk, query-tile, key-block), accumulated in a persistent PSUM bank
  across the whole key loop (a split-PV variant measured slower: sub-512
  row matmuls pay ~90ns fixed overhead each).
- S -> exp -> PV software pipeline: S(next) issues between S(cur) and
  PV(cur) so the PE is not exposed to the exp latency.
"""
import numpy as np
import ml_dtypes

import concourse.bacc as bacc
import concourse.bass as bass
import concourse.mybir as mybir
import concourse.tile as tile
from concourse.bass_utils import run_bass_kernel_spmd
from concourse.masks import make_identity

F32 = mybir.dt.float32
F32R = mybir.dt.float32r
BF16 = mybir.dt.bfloat16
AF = mybir.ActivationFunctionType

B, C, H, W = 4, 512, 48, 48
N = B * H * W            # 9216 tokens
NCORES = 8
Q = N // NCORES          # 1152 queries/keys per core
CB = C // 128            # 4 channel blocks
ST = 384                 # tokens per gather sub-slice / key chunk
NSUB = Q // ST           # 3 subs per core
KB = ST // 128           # 3 key blocks per chunk
QT = 384                 # query tile
NQT = Q // QT            # 3
SHIFT = -40.0
BN_EPS = 1e-5
GRH = CB * ST                    # f32 cols of rhs in gather buf (1536)
GCOL = GRH + (KB * C) // 2       # + xt bf16 as f32 pairs (768) = 2304

_nc_cache = None


class _SafeBacc(bacc.Bacc):
    """Keep matmul waits on the matmul (see v1 note re float32r LDWEIGHTS)."""

    def move_matmul_waits_to_ldweights(self):
        pass


def _build(use_collective=True):
    nc = _SafeBacc("TRN2", target_bir_lowering=False, debug=False,
                   num_devices=NCORES)

    d_q = nc.declare_dram_parameter("d_q", [C, Q], F32, isOutput=False)
    x_q = nc.declare_dram_parameter("x_q", [C, Q], BF16, isOutput=False)
    wts_f = {
        name: nc.declare_dram_parameter(name, [C, C], F32, isOutput=False)
        for name in ["wT_lhs", "wT_rhs"]
    }
    wts_b = {
        name: nc.declare_dram_parameter(name, [C, C], BF16, isOutput=False)
        for name in ["wT_rgb", "wT_dec"]
    }
    vecs = {
        name: nc.declare_dram_parameter(name, [C], F32, isOutput=False)
        for name in ["b_lhs", "b_rhs", "b_rgb", "b_dec", "gamma", "beta"]
    }
    y_out = nc.declare_dram_parameter("y", [C, Q], F32, isOutput=True)

    dq_re = d_q.rearrange("(cb p) n -> p cb n", p=128)
    xq_re = x_q.rearrange("(cb p) n -> p cb n", p=128)
    y_re = y_out.rearrange("(cb p) n -> p cb n", p=128)

    with tile.TileContext(nc) as tc:
        with (
            tc.tile_pool(name="consts", bufs=1) as consts,
            tc.tile_pool(name="chunks", bufs=3) as chunks,
            tc.tile_pool(name="et", bufs=3) as etp,
            tc.tile_pool(name="res", bufs=1) as res,
            tc.tile_pool(name="outp", bufs=2) as outp,
            tc.tile_pool(name="mm", bufs=2, space="PSUM") as mmp,
            tc.tile_pool(name="enh", bufs=1, space="PSUM") as enhp,
            tc.tile_pool(name="den", bufs=1, space="PSUM") as denp,
            tc.tile_pool(name="dram", bufs=1, space="DRAM") as dram,
        ):
            # ---- loads. gpsimd: w_rhs then d_q (critical path for phase A);
            # scalar: x_q + bf16 weights; sync: bias vectors. ----
            w_t = {}
            w = consts.tile([128, CB, C], F32R, tag="w_wT_rhs")
            nc.gpsimd.dma_start(
                out=w[:, :, :],
                in_=wts_f["wT_rhs"].rearrange("(cb p) co -> p cb co", p=128))
            w_t["wT_rhs"] = w
            dq_sb = res.tile([128, CB, Q], F32R)
            nc.gpsimd.dma_start(out=dq_sb[:, :, 0:ST], in_=dq_re[:, :, 0:ST])
            nc.gpsimd.dma_start(out=dq_sb[:, :, ST:Q], in_=dq_re[:, :, ST:Q])
            w = consts.tile([128, CB, C], F32R, tag="w_wT_lhs")
            nc.gpsimd.dma_start(
                out=w[:, :, :],
                in_=wts_f["wT_lhs"].rearrange("(cb p) co -> p cb co", p=128))
            w_t["wT_lhs"] = w
            xq_sb = res.tile([128, CB, Q], BF16)
            for name in ["wT_rgb", "wT_dec"]:
                w = consts.tile([128, CB, C], BF16, tag=f"w_{name}")
                nc.scalar.dma_start(
                    out=w[:, :, :],
                    in_=wts_b[name].rearrange("(cb p) co -> p cb co", p=128))
                w_t[name] = w
            nc.scalar.dma_start(out=xq_sb[:, :, :], in_=xq_re[:, :, :])
            v_t = {}
            for name in vecs:
                v = consts.tile([128, CB], F32, tag=f"v_{name}")
                nc.sync.dma_start(out=v[:, :],
                                  in_=vecs[name].rearrange("(cb p) -> p cb", p=128))
                v_t[name] = v

            shift_t = consts.tile([128, 1], F32)
            nc.vector.memset(shift_t, SHIFT)
            ones_bf = consts.tile([128, 1], BF16)
            nc.vector.memset(ones_bf, 1.0)
            ident_f = consts.tile([128, 128], F32)
            make_identity(nc, ident_f[:, :])
            ident_bf = consts.tile([128, 128], BF16)
            nc.vector.tensor_copy(out=ident_bf[:, :], in_=ident_f[:, :])

            # ---- phase A: own rhs/xt slice, packed AllGather per sub ----
            go = []
            for s in range(NSUB):
                t0 = s * ST
                rhs_self = outp.tile([128, CB, ST], F32, tag="rself")
                for co in range(CB):
                    ps = mmp.tile([128, 512], F32, tag="mm")
                    for ci in range(CB):
                        nc.tensor.matmul(
                            ps[:, :ST],
                            lhsT=w_t["wT_rhs"][:, ci, co * 128:(co + 1) * 128],
                            rhs=dq_sb[:, ci, t0:t0 + ST],
                            start=(ci == 0), stop=(ci == CB - 1))
                    nc.vector.tensor_scalar_add(
                        out=rhs_self[:, co, :], in0=ps[:, :ST],
                        scalar1=v_t["b_rhs"][:, co:co + 1])
                xt_self = outp.tile([128, KB, C], BF16, tag="xself")
                for tb in range(KB):
                    ps = mmp.tile([128, 512], F32, tag="mm")
                    for ci in range(CB):
                        nc.tensor.matmul(
                            ps,
                            lhsT=xq_sb[:, ci, t0 + tb * 128:t0 + (tb + 1) * 128],
                            rhs=w_t["wT_rgb"][:, ci, :],
                            start=(ci == 0), stop=(ci == CB - 1))
                    nc.scalar.copy(out=xt_self[:, tb, :], in_=ps)

                gi_s = dram.tile([128, GCOL], F32, tag=f"gi{s}")
                go_s = dram.tile([NCORES, 128, GCOL], F32, tag=f"go{s}",
                                 addr_space="Shared")
                nc.sync.dma_start(
                    out=gi_s[:, 0:GRH],
                    in_=rhs_self[:, :, :].rearrange("p cb n -> p (cb n)"))
                nc.scalar.dma_start(
                    out=gi_s[:, GRH:GCOL],
                    in_=xt_self[:, :, :].rearrange("p tb c -> p (tb c)")
                    .bitcast(F32))
                if use_collective:
                    nc.gpsimd.collective_compute(
                        "AllGather", mybir.AluOpType.bypass,
                        replica_groups=[list(range(NCORES))],
                        ins=[gi_s.opt()], outs=[go_s.opt()])
                else:
                    nc.gpsimd.dma_start(out=go_s[0], in_=gi_s[:])
                go.append(go_s)

            # ---- lhs for this core's queries ----
            enh_acc = res.tile([128, Q // 128, C], F32)   # [q%128, qb9, ch]
            nc.vector.memset(enh_acc[:, :, :], 0.0)
            lhs_sb = res.tile([128, CB, Q], F32R)
            for qt in range(NQT):
                q0 = qt * QT
                for co in range(CB):
                    ps = mmp.tile([128, 512], F32, tag="mm")
                    for ci in range(CB):
                        nc.tensor.matmul(
                            ps[:, :QT],
                            lhsT=w_t["wT_lhs"][:, ci, co * 128:(co + 1) * 128],
                            rhs=dq_sb[:, ci, q0:q0 + QT],
                            start=(ci == 0), stop=(ci == CB - 1))
                    nc.vector.tensor_scalar_add(
                        out=lhs_sb[:, co, q0:q0 + QT], in0=ps[:, :QT],
                        scalar1=v_t["b_lhs"][:, co:co + 1])

            # ---- main key loop: 24 gathered chunks, S->exp->PV pipelined.
            # Steps are (kc, qt, kb); PV/enh-add for step i-1 issue after
            # S/exp of step i so the PE never waits on the exp latency. ----
            NCH = NSUB * NCORES
            den_ps = denp.tile([128, NQT * 512], F32, tag="den")
            steps = [(kc, qt, kb)
                     for kc in range(NCH)
                     for qt in range(NQT)
                     for kb in range(KB)]

            chunk_tiles = {}

            def load_chunk(kc):
                s, g = kc // NCORES, kc % NCORES
                rhs_t = chunks.tile([128, CB, ST], F32R, tag="rhs")
                nc.gpsimd.dma_start(
                    out=rhs_t[:, :, :],
                    in_=go[s][g, :, 0:GRH]
                    .rearrange("p (cb n) -> p cb n", cb=CB))
                xt_t = chunks.tile([128, KB, C], BF16, tag="xt")
                nc.scalar.dma_start(
                    out=xt_t[:, :, :],
                    in_=go[s][g, :, GRH:GCOL].bitcast(BF16)
                    .rearrange("p (tb c) -> p tb c", tb=KB))
                chunk_tiles[kc] = (rhs_t, xt_t)

            load_chunk(0)
            load_chunk(1)
            groups = {}   # (kc, qt) -> enh_ps tile
            prev = None   # (kc, qt, kb, e_t)

            for i, (kc, qt, kb) in enumerate(steps):
                if qt == 0 and kb == 0 and kc + 2 < NCH:
                    load_chunk(kc + 2)
                rhs_t, _ = chunk_tiles[kc]
                q0 = qt * QT
                st = mmp.tile([128, QT], F32, tag="mm")
                for ci in range(CB):
                    nc.tensor.matmul(
                        st,
                        lhsT=rhs_t[:, ci, kb * 128:(kb + 1) * 128],
                        rhs=lhs_sb[:, ci, q0:q0 + QT],
                        start=(ci == 0), stop=(ci == CB - 1))
                e_t = etp.tile([128, QT], BF16, tag="et")
                nc.scalar.activation(out=e_t, in_=st, func=AF.Exp,
                                     bias=shift_t[:, :], scale=1.0)

                if prev is not None:
                    _pv_and_close(nc, enhp, enh_acc, chunk_tiles, groups,
                                  den_ps, ones_bf, prev)
                prev = (kc, qt, kb, e_t)
                if kb == KB - 1 and qt == NQT - 1 and kc - 1 in chunk_tiles:
                    del chunk_tiles[kc - 1]
            _pv_and_close(nc, enhp, enh_acc, chunk_tiles, groups,
                          den_ps, ones_bf, prev)

            # ---- epilogue ----
            # scatter denominators [1, qt*512+qb*128+p] -> [p, qt*3+qb].
            # SBUF/PSUM APs cannot synthesize a partition dim from free
            # positions (illegal partition step), so bounce through DRAM.
            den_row = consts.tile([128, NQT * 512], F32, tag="denrow")
            nc.scalar.copy(out=den_row[0:1, :], in_=den_ps[0:1, :])
            den_dram = dram.tile([NQT * 512], F32)
            nc.sync.dma_start(out=den_dram[:], in_=den_row[0:1, :])
            rden_pre = consts.tile([128, Q // 128], F32, tag="rdenp")
            for qt in range(NQT):
                nc.sync.dma_start(
                    out=rden_pre[:, qt * NQT:(qt + 1) * NQT],
                    in_=den_dram[qt * 512:qt * 512 + QT]
                    .rearrange("(b p) -> p b", p=128))
            rden = consts.tile([128, Q // 128], F32, tag="rden")
            nc.vector.reciprocal(out=rden[:, :], in_=rden_pre[:, :])

            # normalize, transpose to channel-major, fold b_rgb
            CH_OFF = (0, 128, 256, 384)
            enh_cm = res.tile([128, CB, Q], BF16)
            for qb9 in range(Q // 128):
                en = outp.tile([128, C], BF16, tag="en")
                nc.scalar.activation(out=en, in_=enh_acc[:, qb9, :],
                                     func=AF.Identity,
                                     scale=rden[:, qb9:qb9 + 1])
                for cb in range(CB):
                    tp = mmp.tile([128, 128], BF16, tag="mm")
                    nc.tensor.transpose(
                        tp, en[:, CH_OFF[cb]:CH_OFF[cb] + 128], ident_bf[:, :])
                    nc.vector.tensor_scalar_add(
                        out=enh_cm[:, cb, qb9 * 128:(qb9 + 1) * 128], in0=tp,
                        scalar1=v_t["b_rgb"][:, cb:cb + 1])

            # decoder linear + BN partial sums
            y_sb = res.tile([128, CB, Q], F32)
            psums = consts.tile([128, NQT, 2 * CB], F32, tag="psums")
            junk = outp.tile([128, QT], F32, tag="junk")
            for co in range(CB):
                for qt in range(NQT):
                    ps = mmp.tile([128, QT], F32, tag="mm")
                    for ci in range(CB):
                        nc.tensor.matmul(
                            ps,
                            lhsT=w_t["wT_dec"][:, ci, co * 128:(co + 1) * 128],
                            rhs=enh_cm[:, ci, qt * QT:(qt + 1) * QT],
                            start=(ci == 0), stop=(ci == CB - 1))
                    nc.vector.tensor_scalar_add(
                        out=y_sb[:, co, qt * QT:(qt + 1) * QT], in0=ps,
                        scalar1=v_t["b_dec"][:, co:co + 1])
                    nc.vector.reduce_sum(
                        out=psums[:, qt, co:co + 1],
                        in_=y_sb[:, co, qt * QT:(qt + 1) * QT],
                        axis=mybir.AxisListType.X)
                    nc.vector.tensor_mul(
                        out=junk[:, :],
                        in0=y_sb[:, co, qt * QT:(qt + 1) * QT],
                        in1=y_sb[:, co, qt * QT:(qt + 1) * QT])
                    nc.vector.reduce_sum(
                        out=psums[:, qt, CB + co:CB + co + 1],
                        in_=junk[:, :],
                        axis=mybir.AxisListType.X)

            sums = consts.tile([128, 2 * CB], F32, tag="sums")
            nc.vector.tensor_add(out=sums[:, :], in0=psums[:, 0, :],
                                 in1=psums[:, 1, :])
            nc.vector.tensor_add(out=sums[:, :], in0=sums[:, :],
                                 in1=psums[:, 2, :])

            ar_in = dram.tile([128, 2 * CB], F32)
            ar_out = dram.tile([128, 2 * CB], F32)
            nc.gpsimd.dma_start(out=ar_in[:], in_=sums[:, :])
            if use_collective:
                nc.gpsimd.collective_compute(
                    "AllReduce", mybir.AluOpType.add,
                    replica_groups=[list(range(NCORES))],
                    ins=[ar_in.opt()], outs=[ar_out.opt()])
            else:
                nc.gpsimd.dma_start(out=ar_out[:], in_=ar_in[:])
            gs = consts.tile([128, 2 * CB], F32)
            nc.gpsimd.dma_start(out=gs[:, :], in_=ar_out[:])

            # mean/var -> scale/bias (rsqrt = ACT sqrt + DVE recip + Newton)
            mean_t = consts.tile([128, CB], F32)
            nc.vector.tensor_scalar_mul(out=mean_t[:, :], in0=gs[:, 0:CB],
                                        scalar1=1.0 / N)
            var_t = consts.tile([128, CB], F32)
            nc.vector.tensor_scalar_mul(out=var_t[:, :], in0=gs[:, CB:2 * CB],
                                        scalar1=1.0 / N)
            m2 = consts.tile([128, CB], F32)
            nc.vector.tensor_mul(out=m2[:, :], in0=mean_t[:, :], in1=mean_t[:, :])
            nc.vector.tensor_sub(out=var_t[:, :], in0=var_t[:, :], in1=m2[:, :])
            nc.vector.tensor_scalar_add(out=var_t[:, :], in0=var_t[:, :],
                                        scalar1=BN_EPS)
            sq = consts.tile([128, CB], F32)
            nc.scalar.sqrt(out=sq[:, :], in_=var_t[:, :])
            inv0 = consts.tile([128, CB], F32)
            nc.vector.reciprocal(out=inv0[:, :], in_=sq[:, :])
            t1 = consts.tile([128, CB], F32)
            nc.vector.tensor_mul(out=t1[:, :], in0=var_t[:, :], in1=inv0[:, :])
            nc.vector.tensor_mul(out=t1[:, :], in0=t1[:, :], in1=inv0[:, :])
            nc.vector.tensor_scalar(out=t1[:, :], in0=t1[:, :],
                                    scalar1=-0.5, scalar2=1.5,
                                    op0=mybir.AluOpType.mult,
                                    op1=mybir.AluOpType.add)
            inv_t = consts.tile([128, CB], F32)
            nc.vector.tensor_mul(out=inv_t[:, :], in0=inv0[:, :], in1=t1[:, :])

            scale_t = consts.tile([128, CB], F32)
            nc.vector.tensor_mul(out=scale_t[:, :], in0=inv_t[:, :],
                                 in1=v_t["gamma"][:, :])
            bias2_t = consts.tile([128, CB], F32)
            nc.vector.tensor_mul(out=bias2_t[:, :], in0=mean_t[:, :],
                                 in1=scale_t[:, :])
            nc.vector.tensor_sub(out=bias2_t[:, :], in0=v_t["beta"][:, :],
                                 in1=bias2_t[:, :])

            for cb in range(CB):
                yo = outp.tile([128, Q], F32, tag="yo")
                nc.scalar.activation(out=yo, in_=y_sb[:, cb, :], func=AF.Relu,
                                     scale=scale_t[:, cb:cb + 1],
                                     bias=bias2_t[:, cb:cb + 1])
                nc.sync.dma_start(out=y_re[:, cb, :], in_=yo)

    nc.finalize()
    return nc


def _pv_and_close(nc, enhp, enh_acc, chunk_tiles, groups, den_ps, ones_bf,
                  prev):
    """Issue the PV + den matmuls for a pipelined step; on the last key
    block of a (chunk, query-tile) group, fold the PSUM into enh_acc."""
    kc, qt, kb, e_t = prev
    _, xt_t = chunk_tiles[kc]
    if kb == 0:
        enh_new = enhp.tile([128, NQT, 512], F32, tag="enh")
        groups[(kc, qt)] = enh_new
    enh_ps = groups[(kc, qt)]
    for qb in range(NQT):
        nc.tensor.matmul(
            enh_ps[:, qb, :],
            lhsT=e_t[:, qb * 128:(qb + 1) * 128],
            rhs=xt_t[:, kb, :],
            start=(kb == 0), stop=(kb == KB - 1))
    NCH = NSUB * NCORES
    nc.tensor.matmul(
        den_ps[0:1, qt * 512:qt * 512 + QT],
        lhsT=ones_bf[:, :],
        rhs=e_t[:, :],
        start=(kc == 0 and kb == 0),
        stop=(kc == NCH - 1 and kb == KB - 1))
    if kb == KB - 1:
        nc.vector.tensor_add(
            out=enh_acc[:, qt * NQT:(qt + 1) * NQT, :],
            in0=enh_acc[:, qt * NQT:(qt + 1) * NQT, :],
            in1=enh_ps[:, :, :])
        del groups[(kc, qt)]


def _prepare_in_maps(x, from_depth_estimation, w_rgb, b_rgb, w_lhs, b_lhs,
                     w_rhs, b_rhs, w_dec, b_dec, gamma, beta):
    f32 = np.float32
    bf16 = ml_dtypes.bfloat16
    x_cm = np.asarray(x, dtype=f32).transpose(1, 0, 2, 3).reshape(C, N)
    d_cm = np.asarray(from_depth_estimation, dtype=f32).transpose(1, 0, 2, 3) \
        .reshape(C, N)
    base = {
        "wT_lhs": np.ascontiguousarray(np.asarray(w_lhs, dtype=f32).T),
        "wT_rhs": np.ascontiguousarray(np.asarray(w_rhs, dtype=f32).T),
        "wT_rgb": np.ascontiguousarray(np.asarray(w_rgb, dtype=f32).T
                                       .astype(bf16)),
        "wT_dec": np.ascontiguousarray(np.asarray(w_dec, dtype=f32).T
                                       .astype(bf16)),
        "b_lhs": np.asarray(b_lhs, dtype=f32),
        "b_rhs": np.asarray(b_rhs, dtype=f32),
        "b_rgb": np.asarray(b_rgb, dtype=f32),
        "b_dec": np.asarray(b_dec, dtype=f32),
        "gamma": np.asarray(gamma, dtype=f32),
        "beta": np.asarray(beta, dtype=f32),
    }
    in_maps = []
    for i in range(NCORES):
        m = dict(base)
        m["d_q"] = np.ascontiguousarray(d_cm[:, i * Q:(i + 1) * Q])
        m["x_q"] = np.ascontiguousarray(x_cm[:, i * Q:(i + 1) * Q]
                                        .astype(bf16))
        in_maps.append(m)
    return in_maps


def _assemble(results):
    out = np.empty((B, C, H, W), dtype=np.float32)
    rows = H // (NCORES // B)
    for i in range(NCORES):
        b, half = i // 2, i % 2
        out[b, :, half * rows:(half + 1) * rows, :] = (
            results[i]["y"].reshape(C, rows, W))
    return out


def kernel(x, from_depth_estimation, w_rgb, b_rgb, w_lhs, b_lhs, w_rhs, b_rhs,
           w_dec, b_dec, gamma, beta):
    global _nc_cache
    in_maps = _prepare_in_maps(x, from_depth_estimation, w_rgb, b_rgb, w_lhs,
                               b_lhs, w_rhs, b_rhs, w_dec, b_dec, gamma, beta)
    if _nc_cache is None:
        _nc_cache = _build()
    res = run_bass_kernel_spmd(_nc_cache, in_maps, list(range(NCORES)))
    return _assemble(res.results)


# revision 19
# speedup vs baseline: 1.1670x; 1.1670x over previous
"""DepthGatedModule kernel v3 for 8 Trainium2 NeuronCores (Bass/Tile).

vs v2:
- rhs/xt linears computed once per token slice and AllGathered (packed
  f32 rhs + bf16 xt in one DRAM buffer per 384-token sub-slice) instead
  of recomputed 8x. Key loop streams gathered tiles; no in-loop linears.
- softmax denominator folded into the PV matmul: xt carries a ones
  column (layout [ch0..255 | 1 | ch256..511 | pad]), PV splits into
  257+256-wide matmuls sharing the e stationary. den lands per-partition
  in the enh accumulator -- no separate den matmuls, no DRAM bounce.
- S -> exp -> PV software pipeline: S(next) issues between S(cur) and
  PV(cur) so the PE is not exposed to the exp latency.
"""
import numpy as np
import ml_dtypes

import concourse.bacc as bacc
import concourse.bass as bass
import concourse.mybir as mybir
import concourse.tile as tile
from concourse.bass_utils import run_bass_kernel_spmd
from concourse.masks import make_identity

F32 = mybir.dt.float32
F32R = mybir.dt.float32r
BF16 = mybir.dt.bfloat16
AF = mybir.ActivationFunctionType

B, C, H, W = 4, 512, 48, 48
N = B * H * W            # 9216 tokens
NCORES = 8
Q = N // NCORES          # 1152 queries/keys per core
CB = C // 128            # 4 channel blocks
ST = 384                 # tokens per gather sub-slice / key chunk
NSUB = Q // ST           # 3 subs per core
KB = ST // 128           # 3 key blocks per chunk
QT = 384                 # query tile
NQT = Q // QT            # 3
XW = C + 2               # xt row width: 256ch | 1 | 256ch | pad
HB = XW // 2             # 257: half-block width
SHIFT = -40.0
BN_EPS = 1e-5
GRH = CB * ST                    # f32 cols of rhs in gather buf (1536)
GCOL = GRH + (KB * XW) // 2      # + xt bf16 as f32 pairs (771) = 2307

_nc_cache = None


class _SafeBacc(bacc.Bacc):
    """Keep matmul waits on the matmul (see v1 note re float32r LDWEIGHTS)."""

    def move_matmul_waits_to_ldweights(self):
        pass


def _build(use_collective=True):
    nc = _SafeBacc("TRN2", target_bir_lowering=False, debug=False,
                   num_devices=NCORES)

    d_q = nc.declare_dram_parameter("d_q", [C, Q], F32, isOutput=False)
    x_q = nc.declare_dram_parameter("x_q", [C, Q], BF16, isOutput=False)
    wts_f = {
        name: nc.declare_dram_parameter(name, [C, C], F32, isOutput=False)
        for name in ["wT_lhs", "wT_rhs"]
    }
    wts_b = {
        name: nc.declare_dram_parameter(name, [C, C], BF16, isOutput=False)
        for name in ["wT_rgb", "wT_dec"]
    }
    vecs = {
        name: nc.declare_dram_parameter(name, [C], F32, isOutput=False)
        for name in ["b_lhs", "b_rhs", "b_rgb", "b_dec", "gamma", "beta"]
    }
    y_out = nc.declare_dram_parameter("y", [C, Q], F32, isOutput=True)

    dq_re = d_q.rearrange("(cb p) n -> p cb n", p=128)
    xq_re = x_q.rearrange("(cb p) n -> p cb n", p=128)
    y_re = y_out.rearrange("(cb p) n -> p cb n", p=128)

    with tile.TileContext(nc) as tc:
        with (
            tc.tile_pool(name="consts", bufs=1) as consts,
            tc.tile_pool(name="chunks", bufs=4) as chunks,
            tc.tile_pool(name="et", bufs=3) as etp,
            tc.tile_pool(name="res", bufs=1) as res,
            tc.tile_pool(name="outp", bufs=2) as outp,
            tc.tile_pool(name="mm", bufs=2, space="PSUM") as mmp,
            tc.tile_pool(name="enh", bufs=1, space="PSUM") as enhp,
            tc.tile_pool(name="dram", bufs=1, space="DRAM") as dram,
        ):
            # ---- loads. gpsimd: w_rhs then d_q (critical path for phase A);
            # scalar: x_q + bf16 weights; sync: bias vectors. ----
            w_t = {}
            w = consts.tile([128, CB, C], F32R, tag="w_wT_rhs")
            nc.gpsimd.dma_start(
                out=w[:, :, :],
                in_=wts_f["wT_rhs"].rearrange("(cb p) co -> p cb co", p=128))
            w_t["wT_rhs"] = w
            dq_sb = res.tile([128, CB, Q], F32R)
            nc.gpsimd.dma_start(out=dq_sb[:, :, 0:ST], in_=dq_re[:, :, 0:ST])
            nc.gpsimd.dma_start(out=dq_sb[:, :, ST:Q], in_=dq_re[:, :, ST:Q])
            w = consts.tile([128, CB, C], F32R, tag="w_wT_lhs")
            nc.gpsimd.dma_start(
                out=w[:, :, :],
                in_=wts_f["wT_lhs"].rearrange("(cb p) co -> p cb co", p=128))
            w_t["wT_lhs"] = w
            xq_sb = res.tile([128, CB, Q], BF16)
            for name in ["wT_rgb", "wT_dec"]:
                w = consts.tile([128, CB, C], BF16, tag=f"w_{name}")
                nc.scalar.dma_start(
                    out=w[:, :, :],
                    in_=wts_b[name].rearrange("(cb p) co -> p cb co", p=128))
                w_t[name] = w
            nc.scalar.dma_start(out=xq_sb[:, :, :], in_=xq_re[:, :, :])
            v_t = {}
            for name in vecs:
                v = consts.tile([128, CB], F32, tag=f"v_{name}")
                nc.sync.dma_start(out=v[:, :],
                                  in_=vecs[name].rearrange("(cb p) -> p cb", p=128))
                v_t[name] = v

            shift_t = consts.tile([128, 1], F32)
            nc.vector.memset(shift_t, SHIFT)
            ident_f = consts.tile([128, 128], F32)
            make_identity(nc, ident_f[:, :])
            ident_bf = consts.tile([128, 128], BF16)
            nc.vector.tensor_copy(out=ident_bf[:, :], in_=ident_f[:, :])

            # ---- phase A: own rhs/xt slice, packed AllGather per sub ----
            go = []
            for s in range(NSUB):
                t0 = s * ST
                rhs_self = outp.tile([128, CB, ST], F32, tag="rself")
                for co in range(CB):
                    ps = mmp.tile([128, 512], F32, tag="mm")
                    for ci in range(CB):
                        nc.tensor.matmul(
                            ps[:, :ST],
                            lhsT=w_t["wT_rhs"][:, ci, co * 128:(co + 1) * 128],
                            rhs=dq_sb[:, ci, t0:t0 + ST],
                            start=(ci == 0), stop=(ci == CB - 1))
                    nc.vector.tensor_scalar_add(
                        out=rhs_self[:, co, :], in0=ps[:, :ST],
                        scalar1=v_t["b_rhs"][:, co:co + 1])
                xt_self = outp.tile([128, KB, XW], BF16, tag="xself")
                nc.vector.memset(xt_self[:, :, 256:257], 1.0)
                for tb in range(KB):
                    ps = mmp.tile([128, 512], F32, tag="mm")
                    for ci in range(CB):
                        nc.tensor.matmul(
                            ps,
                            lhsT=xq_sb[:, ci, t0 + tb * 128:t0 + (tb + 1) * 128],
                            rhs=w_t["wT_rgb"][:, ci, :],
                            start=(ci == 0), stop=(ci == CB - 1))
                    nc.scalar.copy(out=xt_self[:, tb, 0:256], in_=ps[:, 0:256])
                    nc.scalar.copy(out=xt_self[:, tb, HB:HB + 256],
                                   in_=ps[:, 256:512])

                gi_s = dram.tile([128, GCOL], F32, tag=f"gi{s}")
                go_s = dram.tile([NCORES, 128, GCOL], F32, tag=f"go{s}",
                                 addr_space="Shared")
                nc.sync.dma_start(
                    out=gi_s[:, 0:GRH],
                    in_=rhs_self[:, :, :].rearrange("p cb n -> p (cb n)"))
                nc.scalar.dma_start(
                    out=gi_s[:, GRH:GCOL],
                    in_=xt_self[:, :, :].rearrange("p tb c -> p (tb c)")
                    .bitcast(F32))
                if use_collective:
                    nc.gpsimd.collective_compute(
                        "AllGather", mybir.AluOpType.bypass,
                        replica_groups=[list(range(NCORES))],
                        ins=[gi_s.opt()], outs=[go_s.opt()])
                else:
                    nc.gpsimd.dma_start(out=go_s[0], in_=gi_s[:])
                go.append(go_s)

            # ---- lhs for this core's queries ----
            enh_acc = res.tile([128, Q // 128, XW], F32)  # [q%128, qb9, ch|den]
            nc.vector.memset(enh_acc[:, :, :], 0.0)
            lhs_sb = res.tile([128, CB, Q], F32R)
            for qt in range(NQT):
                q0 = qt * QT
                for co in range(CB):
                    ps = mmp.tile([128, 512], F32, tag="mm")
                    for ci in range(CB):
                        nc.tensor.matmul(
                            ps[:, :QT],
                            lhsT=w_t["wT_lhs"][:, ci, co * 128:(co + 1) * 128],
                            rhs=dq_sb[:, ci, q0:q0 + QT],
                            start=(ci == 0), stop=(ci == CB - 1))
                    nc.vector.tensor_scalar_add(
                        out=lhs_sb[:, co, q0:q0 + QT], in0=ps[:, :QT],
                        scalar1=v_t["b_lhs"][:, co:co + 1])

            # ---- main key loop: 24 gathered chunks, S->exp->PV pipelined.
            # Steps are (kc, qt, kb); PV/enh-add for step i-1 issue after
            # S/exp of step i so the PE never waits on the exp latency. ----
            NCH = NSUB * NCORES
            steps = [(kc, qt, kb)
                     for kc in range(NCH)
                     for qt in range(NQT)
                     for kb in range(KB)]

            chunk_tiles = {}

            def load_chunk(kc):
                s, g = kc // NCORES, kc % NCORES
                rhs_t = chunks.tile([128, CB, ST], F32R, tag="rhs")
                nc.gpsimd.dma_start(
                    out=rhs_t[:, :, :],
                    in_=go[s][g, :, 0:GRH]
                    .rearrange("p (cb n) -> p cb n", cb=CB))
                xt_t = chunks.tile([128, KB, XW], BF16, tag="xt")
                nc.scalar.dma_start(
                    out=xt_t[:, :, :],
                    in_=go[s][g, :, GRH:GCOL].bitcast(BF16)
                    .rearrange("p (tb c) -> p tb c", tb=KB))
                chunk_tiles[kc] = (rhs_t, xt_t)

            load_chunk(0)
            load_chunk(1)
            groups = {}   # (kc, qt) -> enh_ps tile
            prev = None   # (kc, qt, kb, e_t)

            for i, (kc, qt, kb) in enumerate(steps):
                if qt == 0 and kb == 0 and kc + 2 < NCH:
                    load_chunk(kc + 2)
                rhs_t, _ = chunk_tiles[kc]
                q0 = qt * QT
                st = mmp.tile([128, QT], F32, tag="mm")
                for ci in range(CB):
                    nc.tensor.matmul(
                        st,
                        lhsT=rhs_t[:, ci, kb * 128:(kb + 1) * 128],
                        rhs=lhs_sb[:, ci, q0:q0 + QT],
                        start=(ci == 0), stop=(ci == CB - 1))
                e_t = etp.tile([128, QT], BF16, tag="et")
                nc.scalar.activation(out=e_t, in_=st, func=AF.Exp,
                                     bias=shift_t[:, :], scale=1.0)

                if prev is not None:
                    _pv_and_close(nc, enhp, enh_acc, chunk_tiles, groups, prev)
                prev = (kc, qt, kb, e_t)
                if kb == KB - 1 and qt == NQT - 1 and kc - 1 in chunk_tiles:
                    del chunk_tiles[kc - 1]
            _pv_and_close(nc, enhp, enh_acc, chunk_tiles, groups, prev)

            # ---- epilogue ----
            # den sits at enh_acc[:, qb9, 256]; already per-partition.
            rden = consts.tile([128, Q // 128], F32, tag="rden")
            nc.vector.reciprocal(out=rden[:, :], in_=enh_acc[:, :, 256])

            # normalize, transpose to channel-major, fold b_rgb
            CH_OFF = (0, 128, HB, HB + 128)
            enh_cm = res.tile([128, CB, Q], BF16)
            for qb9 in range(Q // 128):
                en = outp.tile([128, XW], BF16, tag="en")
                nc.scalar.activation(out=en, in_=enh_acc[:, qb9, :],
                                     func=AF.Identity,
                                     scale=rden[:, qb9:qb9 + 1])
                for cb in range(CB):
                    tp = mmp.tile([128, 128], BF16, tag="mm")
                    nc.tensor.transpose(
                        tp, en[:, CH_OFF[cb]:CH_OFF[cb] + 128], ident_bf[:, :])
                    nc.vector.tensor_scalar_add(
                        out=enh_cm[:, cb, qb9 * 128:(qb9 + 1) * 128], in0=tp,
                        scalar1=v_t["b_rgb"][:, cb:cb + 1])

            # decoder linear + BN partial sums
            y_sb = res.tile([128, CB, Q], F32)
            psums = consts.tile([128, NQT, 2 * CB], F32, tag="psums")
            junk = outp.tile([128, QT], F32, tag="junk")
            for co in range(CB):
                for qt in range(NQT):
                    ps = mmp.tile([128, QT], F32, tag="mm")
                    for ci in range(CB):
                        nc.tensor.matmul(
                            ps,
                            lhsT=w_t["wT_dec"][:, ci, co * 128:(co + 1) * 128],
                            rhs=enh_cm[:, ci, qt * QT:(qt + 1) * QT],
                            start=(ci == 0), stop=(ci == CB - 1))
                    nc.vector.tensor_scalar_add(
                        out=y_sb[:, co, qt * QT:(qt + 1) * QT], in0=ps,
                        scalar1=v_t["b_dec"][:, co:co + 1])
                    nc.vector.reduce_sum(
                        out=psums[:, qt, co:co + 1],
                        in_=y_sb[:, co, qt * QT:(qt + 1) * QT],
                        axis=mybir.AxisListType.X)
                    nc.vector.tensor_mul(
                        out=junk[:, :],
                        in0=y_sb[:, co, qt * QT:(qt + 1) * QT],
                        in1=y_sb[:, co, qt * QT:(qt + 1) * QT])
                    nc.vector.reduce_sum(
                        out=psums[:, qt, CB + co:CB + co + 1],
                        in_=junk[:, :],
                        axis=mybir.AxisListType.X)

            sums = consts.tile([128, 2 * CB], F32, tag="sums")
            nc.vector.tensor_add(out=sums[:, :], in0=psums[:, 0, :],
                                 in1=psums[:, 1, :])
            nc.vector.tensor_add(out=sums[:, :], in0=sums[:, :],
                                 in1=psums[:, 2, :])

            ar_in = dram.tile([128, 2 * CB], F32)
            ar_out = dram.tile([128, 2 * CB], F32)
            nc.gpsimd.dma_start(out=ar_in[:], in_=sums[:, :])
            if use_collective:
                nc.gpsimd.collective_compute(
                    "AllReduce", mybir.AluOpType.add,
                    replica_groups=[list(range(NCORES))],
                    ins=[ar_in.opt()], outs=[ar_out.opt()])
            else:
                nc.gpsimd.dma_start(out=ar_out[:], in_=ar_in[:])
            gs = consts.tile([128, 2 * CB], F32)
            nc.gpsimd.dma_start(out=gs[:, :], in_=ar_out[:])

            # mean/var -> scale/bias (rsqrt = ACT sqrt + DVE recip + Newton)
            mean_t = consts.tile([128, CB], F32)
            nc.vector.tensor_scalar_mul(out=mean_t[:, :], in0=gs[:, 0:CB],
                                        scalar1=1.0 / N)
            var_t = consts.tile([128, CB], F32)
            nc.vector.tensor_scalar_mul(out=var_t[:, :], in0=gs[:, CB:2 * CB],
                                        scalar1=1.0 / N)
            m2 = consts.tile([128, CB], F32)
            nc.vector.tensor_mul(out=m2[:, :], in0=mean_t[:, :], in1=mean_t[:, :])
            nc.vector.tensor_sub(out=var_t[:, :], in0=var_t[:, :], in1=m2[:, :])
            nc.vector.tensor_scalar_add(out=var_t[:, :], in0=var_t[:, :],
                                        scalar1=BN_EPS)
            sq = consts.tile([128, CB], F32)
            nc.scalar.sqrt(out=sq[:, :], in_=var_t[:, :])
            inv0 = consts.tile([128, CB], F32)
            nc.vector.reciprocal(out=inv0[:, :], in_=sq[:, :])
            t1 = consts.tile([128, CB], F32)
            nc.vector.tensor_mul(out=t1[:, :], in0=var_t[:, :], in1=inv0[:, :])
            nc.vector.tensor_mul(out=t1[:, :], in0=t1[:, :], in1=inv0[:, :])
            nc.vector.tensor_scalar(out=t1[:, :], in0=t1[:, :],
                                    scalar1=-0.5, scalar2=1.5,
                                    op0=mybir.AluOpType.mult,
                                    op1=mybir.AluOpType.add)
            inv_t = consts.tile([128, CB], F32)
            nc.vector.tensor_mul(out=inv_t[:, :], in0=inv0[:, :], in1=t1[:, :])

            scale_t = consts.tile([128, CB], F32)
            nc.vector.tensor_mul(out=scale_t[:, :], in0=inv_t[:, :],
                                 in1=v_t["gamma"][:, :])
            bias2_t = consts.tile([128, CB], F32)
            nc.vector.tensor_mul(out=bias2_t[:, :], in0=mean_t[:, :],
                                 in1=scale_t[:, :])
            nc.vector.tensor_sub(out=bias2_t[:, :], in0=v_t["beta"][:, :],
                                 in1=bias2_t[:, :])

            for cb in range(CB):
                yo = outp.tile([128, Q], F32, tag="yo")
                nc.scalar.activation(out=yo, in_=y_sb[:, cb, :], func=AF.Relu,
                                     scale=scale_t[:, cb:cb + 1],
                                     bias=bias2_t[:, cb:cb + 1])
                nc.sync.dma_start(out=y_re[:, cb, :], in_=yo)

    nc.finalize()
    return nc


def _pv_and_close(nc, enhp, enh_acc, chunk_tiles, groups, prev):
    """Issue the split PV matmuls for a pipelined step; on the last key
    block of a (chunk, query-tile) group, fold the PSUM into enh_acc."""
    kc, qt, kb, e_t = prev
    _, xt_t = chunk_tiles[kc]
    if kb == 0:
        enh_new = enhp.tile([128, NQT, 2, 512], F32, tag="enh")
        groups[(kc, qt)] = enh_new
    enh_ps = groups[(kc, qt)]
    for qb in range(NQT):
        eb = e_t[:, qb * 128:(qb + 1) * 128]
        nc.tensor.matmul(
            enh_ps[:, qb, 0, 0:HB],
            lhsT=eb, rhs=xt_t[:, kb, 0:HB],
            start=(kb == 0), stop=(kb == KB - 1))
        nc.tensor.matmul(
            enh_ps[:, qb, 1, 0:256],
            lhsT=eb, rhs=xt_t[:, kb, HB:HB + 256],
            start=(kb == 0), stop=(kb == KB - 1))
    if kb == KB - 1:
        nc.vector.tensor_add(
            out=enh_acc[:, qt * NQT:(qt + 1) * NQT, :]
            .rearrange("p a (b c) -> p a b c", b=2),
            in0=enh_acc[:, qt * NQT:(qt + 1) * NQT, :]
            .rearrange("p a (b c) -> p a b c", b=2),
            in1=enh_ps[:, :, :, 0:HB])
        del groups[(kc, qt)]


def _prepare_in_maps(x, from_depth_estimation, w_rgb, b_rgb, w_lhs, b_lhs,
                     w_rhs, b_rhs, w_dec, b_dec, gamma, beta):
    f32 = np.float32
    bf16 = ml_dtypes.bfloat16
    x_cm = np.asarray(x, dtype=f32).transpose(1, 0, 2, 3).reshape(C, N)
    d_cm = np.asarray(from_depth_estimation, dtype=f32).transpose(1, 0, 2, 3) \
        .reshape(C, N)
    base = {
        "wT_lhs": np.ascontiguousarray(np.asarray(w_lhs, dtype=f32).T),
        "wT_rhs": np.ascontiguousarray(np.asarray(w_rhs, dtype=f32).T),
        "wT_rgb": np.ascontiguousarray(np.asarray(w_rgb, dtype=f32).T
                                       .astype(bf16)),
        "wT_dec": np.ascontiguousarray(np.asarray(w_dec, dtype=f32).T
                                       .astype(bf16)),
        "b_lhs": np.asarray(b_lhs, dtype=f32),
        "b_rhs": np.asarray(b_rhs, dtype=f32),
        "b_rgb": np.asarray(b_rgb, dtype=f32),
        "b_dec": np.asarray(b_dec, dtype=f32),
        "gamma": np.asarray(gamma, dtype=f32),
        "beta": np.asarray(beta, dtype=f32),
    }
    in_maps = []
    for i in range(NCORES):
        m = dict(base)
        m["d_q"] = np.ascontiguousarray(d_cm[:, i * Q:(i + 1) * Q])
        m["x_q"] = np.ascontiguousarray(x_cm[:, i * Q:(i + 1) * Q]
                                        .astype(bf16))
        in_maps.append(m)
    return in_maps


def _assemble(results):
    out = np.empty((B, C, H, W), dtype=np.float32)
    rows = H // (NCORES // B)
    for i in range(NCORES):
        b, half = i // 2, i % 2
        out[b, :, half * rows:(half + 1) * rows, :] = (
            results[i]["y"].reshape(C, rows, W))
    return out


def kernel(x, from_depth_estimation, w_rgb, b_rgb, w_lhs, b_lhs, w_rhs, b_rhs,
           w_dec, b_dec, gamma, beta):
    global _nc_cache
    in_maps = _prepare_in_maps(x, from_depth_estimation, w_rgb, b_rgb, w_lhs,
                               b_lhs, w_rhs, b_rhs, w_dec, b_dec, gamma, beta)
    if _nc_cache is None:
        _nc_cache = _build()
    res = run_bass_kernel_spmd(_nc_cache, in_maps, list(range(NCORES)))
    return _assemble(res.results)
